# revision 2
# baseline (speedup 1.0000x reference)
"""Trainium2 Bass kernel for per-node temporal graph conv (LCN).

Math (matches the reference): for each node v with neighbor list idx[v]
(chain graph: v-1, v, v+1, masked at the ends),
    out[n,o,v,t] = b[v,o] + sum_{k,c,kt} x_pad[n,c,idx[v,k],t+kt] * Wm[v,o,c,k,kt]

Strategy: data-parallel over batch N across 8 cores (2 samples each);
weights/bias replicated. Per sample, x is laid out in SBUF as 13 "node
pair" blocks of 514 columns (512 + 2 temporal zero pads):
    partitions  0- 63: node 2j-1 (odd nodes; block 0 holds a zero ghost)
    partitions 64-127: node 2j   (even nodes)
so one [128, 512] slice at column offset j*514+kt stacks two adjacent
nodes' time-shifted frames on the contraction dim. Outputs are computed
per node pair (v=2m, 2m+1) stacked on the PSUM partition dim (M=128):
6 accumulating bf16 matmuls (3 temporal taps x 2 source blocks) per
pair, with weight blocks pre-scattered host-side so every (v,u) tap
lands in exactly one matmul. Bias is fused into the PSUM->SBUF copy.
"""

import numpy as np
import ml_dtypes

import concourse.bacc as bacc
import concourse.mybir as mybir
from concourse.tile import TileContext
from concourse.bass_utils import run_bass_kernel_spmd

V, K, CIN, COUT, N, T, TK = 25, 3, 64, 64, 16, 512, 3
NCORES = 8
NPER = N // NCORES          # samples per core
TP = T + 2                  # block width incl. temporal zero pads
NB = (V + 1) // 2           # node-pair blocks
NSLOT = TK * V              # distinct [128,128] weight tiles

_BF16 = mybir.dt.bfloat16
_F32 = mybir.dt.float32

_cache = {}


def _build_program():
    nc = bacc.Bacc("TRN2", num_devices=NCORES)
    x_in = nc.dram_tensor("x", [NPER, CIN, V, T], _F32, kind="ExternalInput")
    wl_in = nc.dram_tensor("wl", [128, NSLOT * 128], _BF16, kind="ExternalInput")
    b_in = nc.dram_tensor("bias", [128, NB], _F32, kind="ExternalInput")
    y_out = nc.dram_tensor("y", [NPER, COUT, V, T], _F32, kind="ExternalOutput")

    with TileContext(nc) as tc:
        with (
            tc.tile_pool(name="w", bufs=1) as wp,
            tc.tile_pool(name="x", bufs=1) as xp,
            tc.tile_pool(name="ps", bufs=8, space="PSUM") as pp,
            tc.tile_pool(name="o", bufs=6) as op,
        ):
            wl_sb = wp.tile([128, NSLOT * 128], _BF16, tag="wl")
            nc.sync.dma_start(out=wl_sb[:, :], in_=wl_in[:, :])
            b_sb = wp.tile([128, NB], _F32, tag="bias")
            nc.sync.dma_start(out=b_sb[:, :], in_=b_in[:, :])

            xs = []
            for n in range(NPER):
                xf = xp.tile([128, NB * TP], _F32, tag=f"xf{n}")
                xfr = xf.rearrange("p (b w) -> p b w", w=TP)
                nc.vector.memset(xfr[:, :, 0:1], 0.0)      # left pads (t=-1)
                nc.vector.memset(xfr[:, :, 513:514], 0.0)  # right pads (t=T)
                nc.vector.memset(xf[0:64, 0:TP], 0.0)      # ghost node -1
                # even nodes 2j -> partitions 64-127, block j
                nc.sync.dma_start(out=xfr[64:128, :, 1:513], in_=x_in[n, :, ::2, :])
                # odd nodes 2j-1 -> partitions 0-63, blocks 1..12
                nc.sync.dma_start(out=xfr[0:64, 1:NB, 1:513], in_=x_in[n, :, 1::2, :])
                xsn = xp.tile([128, NB * TP], _BF16, tag=f"xs{n}")
                nc.vector.tensor_copy(out=xsn[:, :], in_=xf[:, :])
                xs.append(xsn)

            for n in range(NPER):
                for m in range(NB):
                    ps = pp.tile([128, 512], _F32)
                    taps = [
                        (kt, mmi)
                        for kt in range(TK)
                        for mmi in range(2)
                        if 2 * m + mmi < V
                    ]
                    for i, (kt, mmi) in enumerate(taps):
                        slot = kt * V + 2 * m + mmi
                        col = (m + mmi) * TP + kt
                        nc.tensor.matmul(
                            ps[:, :],
                            lhsT=wl_sb[:, slot * 128 : (slot + 1) * 128],
                            rhs=xs[n][:, col : col + 512],
                            start=(i == 0),
                            stop=(i == len(taps) - 1),
                        )
                    ot = op.tile([128, 512], _F32)
                    nc.vector.tensor_scalar_add(
                        out=ot[:, :], in0=ps[:, :], scalar1=b_sb[:, m : m + 1]
                    )
                    if 2 * m + 1 < V:
                        dst = y_out[n].rearrange("o v t -> v o t")[2 * m : 2 * m + 2]
                        nc.sync.dma_start(out=dst, in_=ot[:, :])
                    else:
                        nc.sync.dma_start(out=y_out[n, :, V - 1, :], in_=ot[0:64, :])

    nc.compile()
    return nc


def _prep_weights(W, b, idx, mask):
    W = np.asarray(W, np.float32)
    b = np.asarray(b, np.float32)
    idx = np.asarray(idx)
    mask = np.asarray(mask)
    Wm = np.where(mask[:, None, None, :, None], W, 0.0)  # [V,O,C,K,TK]
    W4 = np.zeros((V, V, COUT, CIN, TK), np.float32)
    for v in range(V):
        for k in range(K):
            if mask[v, k]:
                W4[v, idx[v, k]] = Wm[v, :, :, k, :]
    wl = np.zeros((128, NSLOT * 128), np.float32)
    for kt in range(TK):
        for s in range(V):
            m, mmi = s // 2, s % 2
            slot = kt * V + s
            blk = m + mmi
            for uh, u in ((0, 2 * blk - 1), (1, 2 * blk)):
                for vloc in range(2):
                    v = 2 * m + vloc
                    if 0 <= u < V and v < V:
                        # lhsT[64*uh + c, 64*vloc + o] = W4[v,u,o,c,kt]
                        wl[
                            64 * uh : 64 * uh + 64,
                            slot * 128 + 64 * vloc : slot * 128 + 64 * vloc + 64,
                        ] = W4[v, u, :, :, kt].T
    bias = np.zeros((128, NB), np.float32)
    for m in range(NB):
        for vloc in range(2):
            if 2 * m + vloc < V:
                bias[64 * vloc : 64 * vloc + 64, m] = b[2 * m + vloc]
    return wl.astype(ml_dtypes.bfloat16), bias


def _make_in_maps(inputs):
    x = np.ascontiguousarray(np.asarray(inputs["x"], np.float32))
    wl, bias = _prep_weights(inputs["W"], inputs["b"], inputs["idx"], inputs["mask"])
    return [
        {"x": np.ascontiguousarray(x[c * NPER : (c + 1) * NPER]), "wl": wl, "bias": bias}
        for c in range(NCORES)
    ]


def kernel(x, W, b, idx, mask):
    if "nc" not in _cache:
        _cache["nc"] = _build_program()
    nc = _cache["nc"]
    in_maps = _make_in_maps({"x": x, "W": W, "b": b, "idx": idx, "mask": mask})
    res = run_bass_kernel_spmd(nc, in_maps, list(range(NCORES)))
    return np.concatenate([res.results[c]["y"] for c in range(NCORES)], axis=0)



# revision 3
# speedup vs baseline: 1.0564x; 1.0564x over previous
"""Trainium2 Bass kernel for per-node temporal graph conv (LCN) — v3.

Math (matches the reference): for each node v with neighbor list idx[v]
(chain graph: v-1, v, v+1, masked at the ends),
    out[n,o,v,t] = b[v,o] + sum_{k,c,kt} x_pad[n,c,idx[v,k],t+kt] * Wm[v,o,c,k,kt]

Strategy: data-parallel over batch N across 8 cores (2 samples each);
weights/bias replicated. All layout shuffling happens on the host so
every device DMA is a large fully-contiguous transfer (x pre-packed
into the SBUF node-pair layout in bf16, outputs staged in SBUF and
unshuffled on the host).

v4 on top of v3:
 - DMA issue split across both HWDGE queues (x loads on nc.sync,
   weights/bias/output stores on nc.scalar).
 - ~0.2-0.26 MB uniform chunks issued in strict consumption order.
   The 16 SDMA engines round-robin across the 8 HWDGE lanes, so big
   up-front transfers all complete together (late); small ordered
   chunks + lane-reuse stalls at the sequencer approximate just-in-
   time streaming, so the first pair's data lands ~2x earlier and
   later chunks stay just ahead of the matmuls that consume them.
 - no warm-up matmuls: starting real matmuls early on the cold clock
   costs about the same as idling through dummy warm-up, and is
   simpler.

Per node pair (v=2m, 2m+1), outputs live on the PSUM partition dim
(128 = 2 nodes x 64 ch); 6 accumulating bf16 matmuls (3 temporal taps
x 2 source blocks) per pair. Bias is fused into the PSUM->SBUF copy.
"""

import numpy as np
import ml_dtypes

import concourse.bacc as bacc
import concourse.mybir as mybir
from concourse.tile import TileContext
from concourse.bass_utils import run_bass_kernel_spmd

V, K, CIN, COUT, N, T, TK = 25, 3, 64, 64, 16, 512, 3
NCORES = 8
NPER = N // NCORES          # samples per core
TP = T + 2                  # block width incl. temporal zero pads
NB = (V + 1) // 2           # node-pair blocks
Y_BF16 = True               # store outputs as bf16 (host casts back)

_BF16 = mybir.dt.bfloat16
_F32 = mybir.dt.float32

_cache = {}


def _pair_taps(m):
    return [(kt, mmi) for kt in range(TK) for mmi in range(2) if 2 * m + mmi < V]


# slot index for each (m, kt, mmi), pair-major so weights stream in
# the same order the matmuls consume them
_SLOTS = {}
for _m in range(NB):
    for _t in _pair_taps(_m):
        _SLOTS[(_m,) + _t] = len(_SLOTS)
NSLOT = len(_SLOTS)  # 75


def _build_program():
    ydt = _BF16 if Y_BF16 else _F32
    nc = bacc.Bacc("TRN2", num_devices=NCORES)
    xp_in = nc.dram_tensor("xp", [NPER, 128, NB * TP], _BF16, kind="ExternalInput")
    wl_in = nc.dram_tensor("wl", [128, NSLOT * 128], _BF16, kind="ExternalInput")
    b_in = nc.dram_tensor("bias", [128, NB], _F32, kind="ExternalInput")
    y_out = nc.dram_tensor("y", [NPER, 128, NB * T], ydt, kind="ExternalOutput")

    # weight chunks in slot units (pair m starts at slot 6m); first chunk
    # covers pair 0, the rest are uniform ~0.26 MB
    WCHUNKS = [(0, 6)] + [(lo, min(lo + 8, NSLOT)) for lo in range(6, NSLOT, 8)]
    # x chunks per sample, in block units (~0.26 MB each)
    XCHUNKS = [(lo, min(lo + 2, NB)) for lo in range(0, NB, 2)]
    # y store chunks per sample, in pair units (~0.26 MB each)
    YCHUNKS = [(lo, min(lo + 2, NB)) for lo in range(0, NB, 2)]

    with TileContext(nc) as tc:
        with (
            tc.tile_pool(name="w", bufs=1) as wp,
            tc.tile_pool(name="x", bufs=1) as xp,
            tc.tile_pool(name="ps", bufs=8, space="PSUM") as pp,
            tc.tile_pool(name="o", bufs=1) as op,
        ):
            wl_sb = wp.tile([128, NSLOT * 128], _BF16, tag="wl")
            b_sb = wp.tile([128, NB], _F32, tag="bias")
            xs = [
                xp.tile([128, NB * TP], _BF16, tag=f"xs{n}", name=f"xs{n}")
                for n in range(NPER)
            ]
            ys = [
                op.tile([128, NB * T], ydt, tag=f"ys{n}", name=f"ys{n}")
                for n in range(NPER)
            ]

            # weights/bias on the scalar HWDGE queue, x loads on sync —
            # both in consumption order; the 8 DMA sem lanes self-throttle
            nc.scalar.dma_start(
                out=wl_sb[:, 0 : WCHUNKS[0][1] * 128],
                in_=wl_in[:, 0 : WCHUNKS[0][1] * 128],
            )
            nc.scalar.dma_start(out=b_sb[:, :], in_=b_in[:, :])
            for lo, hi in WCHUNKS[1:]:
                nc.scalar.dma_start(
                    out=wl_sb[:, lo * 128 : hi * 128], in_=wl_in[:, lo * 128 : hi * 128]
                )
            for n in range(NPER):
                for lo, hi in XCHUNKS:
                    nc.sync.dma_start(
                        out=xs[n][:, lo * TP : hi * TP],
                        in_=xp_in[n, :, lo * TP : hi * TP],
                    )

            for n in range(NPER):
                ci = 0
                for m in range(NB):
                    ps = pp.tile([128, 512], _F32)
                    taps = _pair_taps(m)
                    for i, (kt, mmi) in enumerate(taps):
                        slot = _SLOTS[(m, kt, mmi)]
                        col = (m + mmi) * TP + kt
                        nc.tensor.matmul(
                            ps[:, :],
                            lhsT=wl_sb[:, slot * 128 : (slot + 1) * 128],
                            rhs=xs[n][:, col : col + 512],
                            start=(i == 0),
                            stop=(i == len(taps) - 1),
                        )
                    nc.vector.tensor_scalar_add(
                        out=ys[n][:, m * T : (m + 1) * T],
                        in0=ps[:, :],
                        scalar1=b_sb[:, m : m + 1],
                    )
                    if ci < len(YCHUNKS) and m + 1 == YCHUNKS[ci][1]:
                        lo, hi = YCHUNKS[ci]
                        nc.scalar.dma_start(
                            out=y_out[n, :, lo * T : hi * T],
                            in_=ys[n][:, lo * T : hi * T],
                        )
                        ci += 1

    nc.compile()
    return nc


def _prep_weights(W, b, idx, mask):
    W = np.asarray(W, np.float32)
    b = np.asarray(b, np.float32)
    idx = np.asarray(idx)
    mask = np.asarray(mask)
    Wm = np.where(mask[:, None, None, :, None], W, 0.0)  # [V,O,C,K,TK]
    W4 = np.zeros((V, V, COUT, CIN, TK), np.float32)
    for v in range(V):
        for k in range(K):
            if mask[v, k]:
                W4[v, idx[v, k]] = Wm[v, :, :, k, :]
    wl = np.zeros((128, NSLOT * 128), np.float32)
    for (m, kt, mmi), slot in _SLOTS.items():
        blk = m + mmi
        for uh, u in ((0, 2 * blk - 1), (1, 2 * blk)):
            for vloc in range(2):
                v = 2 * m + vloc
                if 0 <= u < V and v < V:
                    # lhsT[64*uh + c, 64*vloc + o] = W4[v,u,o,c,kt]
                    wl[
                        64 * uh : 64 * uh + 64,
                        slot * 128 + 64 * vloc : slot * 128 + 64 * vloc + 64,
                    ] = W4[v, u, :, :, kt].T
    bias = np.zeros((128, NB), np.float32)
    for m in range(NB):
        for vloc in range(2):
            if 2 * m + vloc < V:
                bias[64 * vloc : 64 * vloc + 64, m] = b[2 * m + vloc]
    return wl.astype(ml_dtypes.bfloat16), bias


def _pack_x(x):
    # x: [N, CIN, V, T] f32 -> [N, 128, NB, TP] bf16 in node-pair layout
    xb = x.astype(ml_dtypes.bfloat16)
    xp = np.zeros((N, 128, NB, TP), ml_dtypes.bfloat16)
    # even nodes 2j -> partitions 64-127, block j
    xp[:, 64:128, :, 1 : T + 1] = xb[:, :, 0::2, :]
    # odd nodes 2j-1 -> partitions 0-63, blocks 1..12
    xp[:, 0:64, 1:NB, 1 : T + 1] = xb[:, :, 1::2, :]
    return np.ascontiguousarray(xp.reshape(N, 128, NB * TP))


def _unpack_y(yp):
    # yp: [N, 128, NB*T] -> [N, COUT, V, T] f32.
    # Partition p = vloc*64 + o, column = m*T + t, value = out[n,o,2m+vloc,t].
    y4 = np.asarray(yp, np.float32).reshape(N, 2, COUT, NB, T)
    out = np.empty((N, COUT, V, T), np.float32)
    out[:, :, 0::2, :] = y4[:, 0]
    out[:, :, 1::2, :] = y4[:, 1, :, : V // 2]
    return out


def _make_in_maps(inputs):
    x = np.ascontiguousarray(np.asarray(inputs["x"], np.float32))
    wl, bias = _prep_weights(inputs["W"], inputs["b"], inputs["idx"], inputs["mask"])
    xp = _pack_x(x)
    return [
        {
            "xp": np.ascontiguousarray(xp[c * NPER : (c + 1) * NPER]),
            "wl": wl,
            "bias": bias,
        }
        for c in range(NCORES)
    ]


def kernel(x, W, b, idx, mask):
    if "nc" not in _cache:
        _cache["nc"] = _build_program()
    nc = _cache["nc"]
    in_maps = _make_in_maps({"x": x, "W": W, "b": b, "idx": idx, "mask": mask})
    res = run_bass_kernel_spmd(nc, in_maps, list(range(NCORES)))
    yp = np.concatenate([res.results[c]["y"] for c in range(NCORES)], axis=0)
    return _unpack_y(yp)


# revision 4
# speedup vs baseline: 1.0878x; 1.0297x over previous
"""Trainium2 Bass kernel for per-node temporal graph conv (LCN).

Math (matches the reference): for each node v with neighbor list idx[v]
(chain graph: v-1, v, v+1, masked at the ends),
    out[n,o,v,t] = b[v,o] + sum_{k,c,kt} x_pad[n,c,idx[v,k],t+kt] * Wm[v,o,c,k,kt]

Strategy: data-parallel over batch N across 8 cores (2 samples each);
weights/bias replicated. All layout shuffling happens on the host so
every device DMA is a large fully-contiguous transfer (x pre-packed
into the SBUF node-pair layout in bf16, outputs staged in SBUF and
unshuffled on the host).

Scheduling details:
 - DMA issue split across both HWDGE queues (x loads on nc.sync,
   weights/bias/output stores on nc.scalar).
 - ~0.2-0.26 MB uniform chunks issued in strict consumption order.
   The 16 SDMA engines round-robin across the 8 HWDGE lanes, so big
   up-front transfers all complete together (late); small ordered
   chunks + lane-reuse stalls at the sequencer approximate just-in-
   time streaming, so the first pair's data lands ~2x earlier and
   later chunks stay just ahead of the matmuls that consume them.
 - no warm-up matmuls: starting real matmuls early on the cold clock
   costs about the same as idling through dummy warm-up, and is
   simpler.

Per node pair (v=2m, 2m+1), outputs live on the PSUM partition dim
(128 = 2 nodes x 64 ch); 6 accumulating bf16 matmuls (3 temporal taps
x 2 source blocks) per pair. Bias is fused into the PSUM->SBUF copy.
"""

import numpy as np
import ml_dtypes

import concourse.bacc as bacc
import concourse.mybir as mybir
from concourse.tile import TileContext
from concourse.bass_utils import run_bass_kernel_spmd

V, K, CIN, COUT, N, T, TK = 25, 3, 64, 64, 16, 512, 3
NCORES = 8
NPER = N // NCORES          # samples per core
TP = T + 2                  # block width incl. temporal zero pads
NB = (V + 1) // 2           # node-pair blocks
Y_BF16 = True               # store outputs as bf16 (host casts back)

_BF16 = mybir.dt.bfloat16
_F32 = mybir.dt.float32

_cache = {}


def _pair_taps(m):
    # mmi-major: the first three matmuls of pair m read only source block
    # m, so they can start before block m+1 has landed
    return [(kt, mmi) for mmi in range(2) for kt in range(TK) if 2 * m + mmi < V]


# slot index for each (m, kt, mmi), pair-major so weights stream in
# the same order the matmuls consume them
_SLOTS = {}
for _m in range(NB):
    for _t in _pair_taps(_m):
        _SLOTS[(_m,) + _t] = len(_SLOTS)
NSLOT = len(_SLOTS)  # 75


def _build_program():
    ydt = _BF16 if Y_BF16 else _F32
    nc = bacc.Bacc("TRN2", num_devices=NCORES)
    xp_in = nc.dram_tensor("xp", [NPER, 128, NB * TP], _BF16, kind="ExternalInput")
    wl_in = nc.dram_tensor("wl", [128, NSLOT * 128], _BF16, kind="ExternalInput")
    b_in = nc.dram_tensor("bias", [128, NB], _F32, kind="ExternalInput")
    y_out = nc.dram_tensor("y", [NPER, 128, NB * T], ydt, kind="ExternalOutput")

    # weight chunks in slot units (pair m starts at slot 6m); first chunk
    # covers pair 0, the rest are uniform ~0.26 MB
    WCHUNKS = [(0, 6)] + [(lo, min(lo + 8, NSLOT)) for lo in range(6, NSLOT, 8)]
    # x chunks per sample, in block units; 1-block first chunk so pair
    # 0's first matmuls start as early as possible
    XCHUNKS = [(0, 1)] + [(lo, min(lo + 2, NB)) for lo in range(1, NB, 2)]
    # y store chunks per sample, in pair units (~0.26 MB each)
    YCHUNKS = [(lo, min(lo + 2, NB)) for lo in range(0, NB, 2)]

    with TileContext(nc) as tc:
        with (
            tc.tile_pool(name="w", bufs=1) as wp,
            tc.tile_pool(name="x", bufs=1) as xp,
            tc.tile_pool(name="ps", bufs=8, space="PSUM") as pp,
            tc.tile_pool(name="o", bufs=1) as op,
        ):
            wl_sb = wp.tile([128, NSLOT * 128], _BF16, tag="wl")
            b_sb = wp.tile([128, NB], _F32, tag="bias")
            xs = [
                xp.tile([128, NB * TP], _BF16, tag=f"xs{n}", name=f"xs{n}")
                for n in range(NPER)
            ]
            ys = [
                op.tile([128, NB * T], ydt, tag=f"ys{n}", name=f"ys{n}")
                for n in range(NPER)
            ]

            # weights/bias on the scalar HWDGE queue, x loads on sync —
            # both in consumption order; the 8 DMA sem lanes self-throttle
            nc.scalar.dma_start(
                out=wl_sb[:, 0 : WCHUNKS[0][1] * 128],
                in_=wl_in[:, 0 : WCHUNKS[0][1] * 128],
            )
            nc.scalar.dma_start(out=b_sb[:, :], in_=b_in[:, :])
            for lo, hi in WCHUNKS[1:]:
                nc.scalar.dma_start(
                    out=wl_sb[:, lo * 128 : hi * 128], in_=wl_in[:, lo * 128 : hi * 128]
                )
            for n in range(NPER):
                for lo, hi in XCHUNKS:
                    nc.sync.dma_start(
                        out=xs[n][:, lo * TP : hi * TP],
                        in_=xp_in[n, :, lo * TP : hi * TP],
                    )

            for n in range(NPER):
                ci = 0
                for m in range(NB):
                    ps = pp.tile([128, 512], _F32)
                    taps = _pair_taps(m)
                    for i, (kt, mmi) in enumerate(taps):
                        slot = _SLOTS[(m, kt, mmi)]
                        col = (m + mmi) * TP + kt
                        nc.tensor.matmul(
                            ps[:, :],
                            lhsT=wl_sb[:, slot * 128 : (slot + 1) * 128],
                            rhs=xs[n][:, col : col + 512],
                            start=(i == 0),
                            stop=(i == len(taps) - 1),
                        )
                    nc.vector.tensor_scalar_add(
                        out=ys[n][:, m * T : (m + 1) * T],
                        in0=ps[:, :],
                        scalar1=b_sb[:, m : m + 1],
                    )
                    if ci < len(YCHUNKS) and m + 1 == YCHUNKS[ci][1]:
                        lo, hi = YCHUNKS[ci]
                        nc.scalar.dma_start(
                            out=y_out[n, :, lo * T : hi * T],
                            in_=ys[n][:, lo * T : hi * T],
                        )
                        ci += 1

    nc.compile()
    return nc


def _prep_weights(W, b, idx, mask):
    W = np.asarray(W, np.float32)
    b = np.asarray(b, np.float32)
    idx = np.asarray(idx)
    mask = np.asarray(mask)
    Wm = np.where(mask[:, None, None, :, None], W, 0.0)  # [V,O,C,K,TK]
    W4 = np.zeros((V, V, COUT, CIN, TK), np.float32)
    for v in range(V):
        for k in range(K):
            if mask[v, k]:
                W4[v, idx[v, k]] = Wm[v, :, :, k, :]
    wl = np.zeros((128, NSLOT * 128), np.float32)
    for (m, kt, mmi), slot in _SLOTS.items():
        blk = m + mmi
        for uh, u in ((0, 2 * blk - 1), (1, 2 * blk)):
            for vloc in range(2):
                v = 2 * m + vloc
                if 0 <= u < V and v < V:
                    # lhsT[64*uh + c, 64*vloc + o] = W4[v,u,o,c,kt]
                    wl[
                        64 * uh : 64 * uh + 64,
                        slot * 128 + 64 * vloc : slot * 128 + 64 * vloc + 64,
                    ] = W4[v, u, :, :, kt].T
    bias = np.zeros((128, NB), np.float32)
    for m in range(NB):
        for vloc in range(2):
            if 2 * m + vloc < V:
                bias[64 * vloc : 64 * vloc + 64, m] = b[2 * m + vloc]
    return wl.astype(ml_dtypes.bfloat16), bias


def _pack_x(x):
    # x: [N, CIN, V, T] f32 -> [N, 128, NB, TP] bf16 in node-pair layout
    xb = x.astype(ml_dtypes.bfloat16)
    xp = np.zeros((N, 128, NB, TP), ml_dtypes.bfloat16)
    # even nodes 2j -> partitions 64-127, block j
    xp[:, 64:128, :, 1 : T + 1] = xb[:, :, 0::2, :]
    # odd nodes 2j-1 -> partitions 0-63, blocks 1..12
    xp[:, 0:64, 1:NB, 1 : T + 1] = xb[:, :, 1::2, :]
    return np.ascontiguousarray(xp.reshape(N, 128, NB * TP))


def _unpack_y(yp):
    # yp: [N, 128, NB*T] -> [N, COUT, V, T] f32.
    # Partition p = vloc*64 + o, column = m*T + t, value = out[n,o,2m+vloc,t].
    y4 = np.asarray(yp, np.float32).reshape(N, 2, COUT, NB, T)
    out = np.empty((N, COUT, V, T), np.float32)
    out[:, :, 0::2, :] = y4[:, 0]
    out[:, :, 1::2, :] = y4[:, 1, :, : V // 2]
    return out


def _make_in_maps(inputs):
    x = np.ascontiguousarray(np.asarray(inputs["x"], np.float32))
    wl, bias = _prep_weights(inputs["W"], inputs["b"], inputs["idx"], inputs["mask"])
    xp = _pack_x(x)
    return [
        {
            "xp": np.ascontiguousarray(xp[c * NPER : (c + 1) * NPER]),
            "wl": wl,
            "bias": bias,
        }
        for c in range(NCORES)
    ]


def kernel(x, W, b, idx, mask):
    if "nc" not in _cache:
        _cache["nc"] = _build_program()
    nc = _cache["nc"]
    in_maps = _make_in_maps({"x": x, "W": W, "b": b, "idx": idx, "mask": mask})
    res = run_bass_kernel_spmd(nc, in_maps, list(range(NCORES)))
    yp = np.concatenate([res.results[c]["y"] for c in range(NCORES)], axis=0)
    return _unpack_y(yp)


# revision 5
# speedup vs baseline: 1.0911x; 1.0030x over previous
"""Trainium2 Bass kernel for per-node temporal graph conv (LCN) — v3.

Math (matches the reference): for each node v with neighbor list idx[v]
(chain graph: v-1, v, v+1, masked at the ends),
    out[n,o,v,t] = b[v,o] + sum_{k,c,kt} x_pad[n,c,idx[v,k],t+kt] * Wm[v,o,c,k,kt]

Strategy: data-parallel over batch N across 8 cores (2 samples each);
weights/bias replicated. All layout shuffling happens on the host so
every device DMA is a large fully-contiguous transfer (x pre-packed
into the SBUF node-pair layout in bf16, outputs staged in SBUF and
unshuffled on the host).

v4 on top of v3:
 - DMA issue split across both HWDGE queues (x loads on nc.sync,
   weights/bias/output stores on nc.scalar).
 - ~0.2-0.26 MB uniform chunks issued in strict consumption order.
   The 16 SDMA engines round-robin across the 8 HWDGE lanes, so big
   up-front transfers all complete together (late); small ordered
   chunks + lane-reuse stalls at the sequencer approximate just-in-
   time streaming, so the first pair's data lands ~2x earlier and
   later chunks stay just ahead of the matmuls that consume them.
 - no warm-up matmuls: starting real matmuls early on the cold clock
   costs about the same as idling through dummy warm-up, and is
   simpler.

Per node pair (v=2m, 2m+1), outputs live on the PSUM partition dim
(128 = 2 nodes x 64 ch); 6 accumulating bf16 matmuls (3 temporal taps
x 2 source blocks) per pair. Bias is fused into the PSUM->SBUF copy.
"""

import numpy as np
import ml_dtypes

import concourse.bacc as bacc
import concourse.mybir as mybir
from concourse.tile import TileContext
from concourse.bass_utils import run_bass_kernel_spmd

V, K, CIN, COUT, N, T, TK = 25, 3, 64, 64, 16, 512, 3
NCORES = 8
NPER = N // NCORES          # samples per core
TP = T + 2                  # block width incl. temporal zero pads
NB = (V + 1) // 2           # node-pair blocks
Y_BF16 = True               # store outputs as bf16 (host casts back)

_BF16 = mybir.dt.bfloat16
_F32 = mybir.dt.float32

_cache = {}


def _pair_taps(m):
    # mmi-major: the first three matmuls of pair m read only source block
    # m, so they can start before block m+1 has landed
    return [(kt, mmi) for mmi in range(2) for kt in range(TK) if 2 * m + mmi < V]


# slot index for each (m, kt, mmi), pair-major so weights stream in
# the same order the matmuls consume them
_SLOTS = {}
for _m in range(NB):
    for _t in _pair_taps(_m):
        _SLOTS[(_m,) + _t] = len(_SLOTS)
NSLOT = len(_SLOTS)  # 75


def _build_program():
    ydt = _BF16 if Y_BF16 else _F32
    nc = bacc.Bacc("TRN2", num_devices=NCORES)
    xp_in = nc.dram_tensor("xp", [NPER, 128, NB * TP], _BF16, kind="ExternalInput")
    wl_in = nc.dram_tensor("wl", [128, NSLOT * 128], _BF16, kind="ExternalInput")
    b_in = nc.dram_tensor("bias", [128, NB], _F32, kind="ExternalInput")
    y_out = nc.dram_tensor("y", [NPER, 128, NB * T], ydt, kind="ExternalOutput")
    warm_out = nc.dram_tensor("warm", [1, 4], _F32, kind="ExternalOutput")

    # weight chunks in slot units (pair m starts at slot 6m); first chunk
    # covers pair 0, the rest are uniform ~0.26 MB
    WCHUNKS = [(0, 6)] + [(lo, min(lo + 8, NSLOT)) for lo in range(6, NSLOT, 8)]
    # x chunks per sample, in block units; 1-block first chunk so pair
    # 0's first matmuls start as early as possible
    XCHUNKS = [(0, 1)] + [(lo, min(lo + 2, NB)) for lo in range(1, NB, 2)]
    # y store chunks per sample, in pair units (~0.26 MB each)
    YCHUNKS = [(lo, min(lo + 2, NB)) for lo in range(0, NB, 2)]

    with TileContext(nc) as tc:
        with (
            tc.tile_pool(name="w", bufs=1) as wp,
            tc.tile_pool(name="x", bufs=1) as xp,
            tc.tile_pool(name="ps", bufs=8, space="PSUM") as pp,
            tc.tile_pool(name="o", bufs=1) as op,
        ):
            # PE HAM warm-up sized to END at/after first-chunk arrival
            # (~11.2us): any idle gap between dummies and real matmuls
            # re-arms the ramp window, so err on the long side — the real
            # matmuls queue behind the dummies on the Tensor FIFO with
            # zero gap and run at full clock from the start
            scratch = wp.tile([128, 640], _BF16, tag="scratch")
            warm_sb = wp.tile([1, 4], _F32, tag="warm_sb")
            wps = pp.tile([128, 512], _F32, tag="ps", name="warm_ps")
            nc.vector.memset(scratch[:, :], 0.0)
            for i in range(5):
                nc.tensor.matmul(
                    wps[:, :],
                    lhsT=scratch[:, 0:128],
                    rhs=scratch[:, 128:640],
                    start=(i == 0),
                    stop=(i == 4),
                )
            nc.vector.tensor_copy(out=warm_sb[:, :], in_=wps[0:1, 0:4])

            wl_sb = wp.tile([128, NSLOT * 128], _BF16, tag="wl")
            b_sb = wp.tile([128, NB], _F32, tag="bias")
            xs = [
                xp.tile([128, NB * TP], _BF16, tag=f"xs{n}", name=f"xs{n}")
                for n in range(NPER)
            ]
            ys = [
                op.tile([128, NB * T], ydt, tag=f"ys{n}", name=f"ys{n}")
                for n in range(NPER)
            ]

            # weights/bias on the scalar HWDGE queue, x loads on sync —
            # both in consumption order; the 8 DMA sem lanes self-throttle
            nc.scalar.dma_start(
                out=wl_sb[:, 0 : WCHUNKS[0][1] * 128],
                in_=wl_in[:, 0 : WCHUNKS[0][1] * 128],
            )
            nc.scalar.dma_start(out=b_sb[:, :], in_=b_in[:, :])
            for lo, hi in WCHUNKS[1:]:
                nc.scalar.dma_start(
                    out=wl_sb[:, lo * 128 : hi * 128], in_=wl_in[:, lo * 128 : hi * 128]
                )
            for n in range(NPER):
                for lo, hi in XCHUNKS:
                    nc.sync.dma_start(
                        out=xs[n][:, lo * TP : hi * TP],
                        in_=xp_in[n, :, lo * TP : hi * TP],
                    )
            nc.sync.dma_start(out=warm_out[:, :], in_=warm_sb[:, :])

            for n in range(NPER):
                ci = 0
                for m in range(NB):
                    ps = pp.tile([128, 512], _F32)
                    taps = _pair_taps(m)
                    for i, (kt, mmi) in enumerate(taps):
                        slot = _SLOTS[(m, kt, mmi)]
                        col = (m + mmi) * TP + kt
                        nc.tensor.matmul(
                            ps[:, :],
                            lhsT=wl_sb[:, slot * 128 : (slot + 1) * 128],
                            rhs=xs[n][:, col : col + 512],
                            start=(i == 0),
                            stop=(i == len(taps) - 1),
                        )
                    nc.vector.tensor_scalar_add(
                        out=ys[n][:, m * T : (m + 1) * T],
                        in0=ps[:, :],
                        scalar1=b_sb[:, m : m + 1],
                    )
                    if ci < len(YCHUNKS) and m + 1 == YCHUNKS[ci][1]:
                        lo, hi = YCHUNKS[ci]
                        nc.scalar.dma_start(
                            out=y_out[n, :, lo * T : hi * T],
                            in_=ys[n][:, lo * T : hi * T],
                        )
                        ci += 1

    nc.compile()
    return nc


def _prep_weights(W, b, idx, mask):
    W = np.asarray(W, np.float32)
    b = np.asarray(b, np.float32)
    idx = np.asarray(idx)
    mask = np.asarray(mask)
    Wm = np.where(mask[:, None, None, :, None], W, 0.0)  # [V,O,C,K,TK]
    W4 = np.zeros((V, V, COUT, CIN, TK), np.float32)
    for v in range(V):
        for k in range(K):
            if mask[v, k]:
                W4[v, idx[v, k]] = Wm[v, :, :, k, :]
    wl = np.zeros((128, NSLOT * 128), np.float32)
    for (m, kt, mmi), slot in _SLOTS.items():
        blk = m + mmi
        for uh, u in ((0, 2 * blk - 1), (1, 2 * blk)):
            for vloc in range(2):
                v = 2 * m + vloc
                if 0 <= u < V and v < V:
                    # lhsT[64*uh + c, 64*vloc + o] = W4[v,u,o,c,kt]
                    wl[
                        64 * uh : 64 * uh + 64,
                        slot * 128 + 64 * vloc : slot * 128 + 64 * vloc + 64,
                    ] = W4[v, u, :, :, kt].T
    bias = np.zeros((128, NB), np.float32)
    for m in range(NB):
        for vloc in range(2):
            if 2 * m + vloc < V:
                bias[64 * vloc : 64 * vloc + 64, m] = b[2 * m + vloc]
    return wl.astype(ml_dtypes.bfloat16), bias


def _pack_x(x):
    # x: [N, CIN, V, T] f32 -> [N, 128, NB, TP] bf16 in node-pair layout
    xb = x.astype(ml_dtypes.bfloat16)
    xp = np.zeros((N, 128, NB, TP), ml_dtypes.bfloat16)
    # even nodes 2j -> partitions 64-127, block j
    xp[:, 64:128, :, 1 : T + 1] = xb[:, :, 0::2, :]
    # odd nodes 2j-1 -> partitions 0-63, blocks 1..12
    xp[:, 0:64, 1:NB, 1 : T + 1] = xb[:, :, 1::2, :]
    return np.ascontiguousarray(xp.reshape(N, 128, NB * TP))


def _unpack_y(yp):
    # yp: [N, 128, NB*T] -> [N, COUT, V, T] f32.
    # Partition p = vloc*64 + o, column = m*T + t, value = out[n,o,2m+vloc,t].
    y4 = np.asarray(yp, np.float32).reshape(N, 2, COUT, NB, T)
    out = np.empty((N, COUT, V, T), np.float32)
    out[:, :, 0::2, :] = y4[:, 0]
    out[:, :, 1::2, :] = y4[:, 1, :, : V // 2]
    return out


def _make_in_maps(inputs):
    x = np.ascontiguousarray(np.asarray(inputs["x"], np.float32))
    wl, bias = _prep_weights(inputs["W"], inputs["b"], inputs["idx"], inputs["mask"])
    xp = _pack_x(x)
    return [
        {
            "xp": np.ascontiguousarray(xp[c * NPER : (c + 1) * NPER]),
            "wl": wl,
            "bias": bias,
        }
        for c in range(NCORES)
    ]


def kernel(x, W, b, idx, mask):
    if "nc" not in _cache:
        _cache["nc"] = _build_program()
    nc = _cache["nc"]
    in_maps = _make_in_maps({"x": x, "W": W, "b": b, "idx": idx, "mask": mask})
    res = run_bass_kernel_spmd(nc, in_maps, list(range(NCORES)))
    yp = np.concatenate([res.results[c]["y"] for c in range(NCORES)], axis=0)
    return _unpack_y(yp)


# revision 7
# speedup vs baseline: 1.1137x; 1.0207x over previous
"""Trainium2 Bass kernel for per-node temporal graph conv (LCN) — v3.

Math (matches the reference): for each node v with neighbor list idx[v]
(chain graph: v-1, v, v+1, masked at the ends),
    out[n,o,v,t] = b[v,o] + sum_{k,c,kt} x_pad[n,c,idx[v,k],t+kt] * Wm[v,o,c,k,kt]

Strategy: data-parallel over batch N across 8 cores (2 samples each);
weights/bias replicated. All layout shuffling happens on the host so
every device DMA is a large fully-contiguous transfer (x pre-packed
into the SBUF node-pair layout in bf16, outputs staged in SBUF and
unshuffled on the host).

v4 on top of v3:
 - DMA issue split across both HWDGE queues (x loads on nc.sync,
   weights/bias/output stores on nc.scalar).
 - ~0.2-0.26 MB uniform chunks issued in strict consumption order.
   The 16 SDMA engines round-robin across the 8 HWDGE lanes, so big
   up-front transfers all complete together (late); small ordered
   chunks + lane-reuse stalls at the sequencer approximate just-in-
   time streaming, so the first pair's data lands ~2x earlier and
   later chunks stay just ahead of the matmuls that consume them.
 - no warm-up matmuls: starting real matmuls early on the cold clock
   costs about the same as idling through dummy warm-up, and is
   simpler.

Per node pair (v=2m, 2m+1), outputs live on the PSUM partition dim
(128 = 2 nodes x 64 ch); 6 accumulating bf16 matmuls (3 temporal taps
x 2 source blocks) per pair. Bias is fused into the PSUM->SBUF copy.
"""

import numpy as np
import ml_dtypes

import concourse.bacc as bacc
import concourse.mybir as mybir
from concourse.tile import TileContext
from concourse.bass_utils import run_bass_kernel_spmd

V, K, CIN, COUT, N, T, TK = 25, 3, 64, 64, 16, 512, 3
NCORES = 8
NPER = N // NCORES          # samples per core
TP = T + 2                  # block width incl. temporal zero pads
NB = (V + 1) // 2           # node-pair blocks
Y_BF16 = True               # store outputs as bf16 (host casts back)

_BF16 = mybir.dt.bfloat16
_F32 = mybir.dt.float32

_cache = {}


def _pair_taps(m):
    # mmi-major: the first three matmuls of pair m read only source block
    # m, so they can start before block m+1 has landed
    return [(kt, mmi) for mmi in range(2) for kt in range(TK) if 2 * m + mmi < V]


# slot index for each (m, kt, mmi), pair-major so weights stream in
# the same order the matmuls consume them
_SLOTS = {}
for _m in range(NB):
    for _t in _pair_taps(_m):
        _SLOTS[(_m,) + _t] = len(_SLOTS)
NSLOT = len(_SLOTS)  # 75


def _build_program():
    ydt = _BF16 if Y_BF16 else _F32
    nc = bacc.Bacc("TRN2", num_devices=NCORES)
    xp_in = nc.dram_tensor("xp", [NPER, 128, NB * TP], _BF16, kind="ExternalInput")
    wl_in = nc.dram_tensor("wl", [128, NSLOT * 128], _BF16, kind="ExternalInput")
    b_in = nc.dram_tensor("bias", [128, NB], _F32, kind="ExternalInput")
    y_out = nc.dram_tensor("y", [NPER, 128, NB * T], ydt, kind="ExternalOutput")
    warm_out = nc.dram_tensor("warm", [1, 4], _F32, kind="ExternalOutput")

    # weight chunks in slot units (pair m starts at slot 6m); first chunk
    # covers pair 0, the rest are uniform ~0.26 MB
    WCHUNKS = [(0, 6), (6, 12)] + [
        (lo, min(lo + 8, NSLOT)) for lo in range(12, NSLOT, 8)
    ]
    # x chunks per sample, in block units; 1-block first chunk so pair
    # 0's first matmuls start as early as possible
    XCHUNKS = [(0, 1)] + [(lo, min(lo + 2, NB)) for lo in range(1, NB, 2)]
    # y store chunks per sample, in pair units (~0.26 MB each)
    YCHUNKS = [(lo, min(lo + 2, NB)) for lo in range(0, NB, 2)]

    with TileContext(nc) as tc:
        with (
            tc.tile_pool(name="w", bufs=1) as wp,
            tc.tile_pool(name="x", bufs=1) as xp,
            tc.tile_pool(name="ps", bufs=8, space="PSUM") as pp,
            tc.tile_pool(name="o", bufs=1) as op,
        ):
            # PE HAM warm-up sized to END at/after first-chunk arrival
            # (~11.2us): any idle gap between dummies and real matmuls
            # re-arms the ramp window, so err on the long side — the real
            # matmuls queue behind the dummies on the Tensor FIFO with
            # zero gap and run at full clock from the start
            scratch = wp.tile([128, 640], _BF16, tag="scratch")
            warm_sb = wp.tile([1, 4], _F32, tag="warm_sb")
            wps = pp.tile([128, 512], _F32, tag="ps", name="warm_ps")
            nc.vector.memset(scratch[:, :], 0.0)
            for i in range(5):
                nc.tensor.matmul(
                    wps[:, :],
                    lhsT=scratch[:, 0:128],
                    rhs=scratch[:, 128:640],
                    start=(i == 0),
                    stop=(i == 4),
                )
            nc.vector.tensor_copy(out=warm_sb[:, :], in_=wps[0:1, 0:4])

            wl_sb = wp.tile([128, NSLOT * 128], _BF16, tag="wl")
            b_sb = wp.tile([128, NB], _F32, tag="bias")
            xs = [
                xp.tile([128, NB * TP], _BF16, tag=f"xs{n}", name=f"xs{n}")
                for n in range(NPER)
            ]
            ys = [
                op.tile([128, NB * T], ydt, tag=f"ys{n}", name=f"ys{n}")
                for n in range(NPER)
            ]

            # weights/bias on the scalar HWDGE queue, x loads on sync —
            # both in consumption order; the 8 DMA sem lanes self-throttle
            nc.scalar.dma_start(
                out=wl_sb[:, 0 : WCHUNKS[0][1] * 128],
                in_=wl_in[:, 0 : WCHUNKS[0][1] * 128],
            )
            for ci, (lo, hi) in enumerate(WCHUNKS[1:]):
                nc.scalar.dma_start(
                    out=wl_sb[:, lo * 128 : hi * 128], in_=wl_in[:, lo * 128 : hi * 128]
                )
                if ci == 1:
                    # bias isn't needed until pair 0's PSUM copy (~12.5us);
                    # issuing it after the pair-1/2 weight chunks keeps the
                    # weight stream ahead of the (now warm-from-the-start)
                    # matmul consumption
                    nc.scalar.dma_start(out=b_sb[:, :], in_=b_in[:, :])
            for n in range(NPER):
                for lo, hi in XCHUNKS:
                    nc.sync.dma_start(
                        out=xs[n][:, lo * TP : hi * TP],
                        in_=xp_in[n, :, lo * TP : hi * TP],
                    )
            nc.sync.dma_start(out=warm_out[:, :], in_=warm_sb[:, :])

            for n in range(NPER):
                ci = 0
                for m in range(NB):
                    ps = pp.tile([128, 512], _F32)
                    taps = _pair_taps(m)
                    for i, (kt, mmi) in enumerate(taps):
                        slot = _SLOTS[(m, kt, mmi)]
                        col = (m + mmi) * TP + kt
                        nc.tensor.matmul(
                            ps[:, :],
                            lhsT=wl_sb[:, slot * 128 : (slot + 1) * 128],
                            rhs=xs[n][:, col : col + 512],
                            start=(i == 0),
                            stop=(i == len(taps) - 1),
                        )
                    nc.vector.tensor_scalar_add(
                        out=ys[n][:, m * T : (m + 1) * T],
                        in0=ps[:, :],
                        scalar1=b_sb[:, m : m + 1],
                    )
                    if ci < len(YCHUNKS) and m + 1 == YCHUNKS[ci][1]:
                        lo, hi = YCHUNKS[ci]
                        nc.scalar.dma_start(
                            out=y_out[n, :, lo * T : hi * T],
                            in_=ys[n][:, lo * T : hi * T],
                        )
                        ci += 1

    nc.compile()
    return nc


def _prep_weights(W, b, idx, mask):
    W = np.asarray(W, np.float32)
    b = np.asarray(b, np.float32)
    idx = np.asarray(idx)
    mask = np.asarray(mask)
    Wm = np.where(mask[:, None, None, :, None], W, 0.0)  # [V,O,C,K,TK]
    W4 = np.zeros((V, V, COUT, CIN, TK), np.float32)
    for v in range(V):
        for k in range(K):
            if mask[v, k]:
                W4[v, idx[v, k]] = Wm[v, :, :, k, :]
    wl = np.zeros((128, NSLOT * 128), np.float32)
    for (m, kt, mmi), slot in _SLOTS.items():
        blk = m + mmi
        for uh, u in ((0, 2 * blk - 1), (1, 2 * blk)):
            for vloc in range(2):
                v = 2 * m + vloc
                if 0 <= u < V and v < V:
                    # lhsT[64*uh + c, 64*vloc + o] = W4[v,u,o,c,kt]
                    wl[
                        64 * uh : 64 * uh + 64,
                        slot * 128 + 64 * vloc : slot * 128 + 64 * vloc + 64,
                    ] = W4[v, u, :, :, kt].T
    bias = np.zeros((128, NB), np.float32)
    for m in range(NB):
        for vloc in range(2):
            if 2 * m + vloc < V:
                bias[64 * vloc : 64 * vloc + 64, m] = b[2 * m + vloc]
    return wl.astype(ml_dtypes.bfloat16), bias


def _pack_x(x):
    # x: [N, CIN, V, T] f32 -> [N, 128, NB, TP] bf16 in node-pair layout
    xb = x.astype(ml_dtypes.bfloat16)
    xp = np.zeros((N, 128, NB, TP), ml_dtypes.bfloat16)
    # even nodes 2j -> partitions 64-127, block j
    xp[:, 64:128, :, 1 : T + 1] = xb[:, :, 0::2, :]
    # odd nodes 2j-1 -> partitions 0-63, blocks 1..12
    xp[:, 0:64, 1:NB, 1 : T + 1] = xb[:, :, 1::2, :]
    return np.ascontiguousarray(xp.reshape(N, 128, NB * TP))


def _unpack_y(yp):
    # yp: [N, 128, NB*T] -> [N, COUT, V, T] f32.
    # Partition p = vloc*64 + o, column = m*T + t, value = out[n,o,2m+vloc,t].
    y4 = np.asarray(yp, np.float32).reshape(N, 2, COUT, NB, T)
    out = np.empty((N, COUT, V, T), np.float32)
    out[:, :, 0::2, :] = y4[:, 0]
    out[:, :, 1::2, :] = y4[:, 1, :, : V // 2]
    return out


def _make_in_maps(inputs):
    x = np.ascontiguousarray(np.asarray(inputs["x"], np.float32))
    wl, bias = _prep_weights(inputs["W"], inputs["b"], inputs["idx"], inputs["mask"])
    xp = _pack_x(x)
    return [
        {
            "xp": np.ascontiguousarray(xp[c * NPER : (c + 1) * NPER]),
            "wl": wl,
            "bias": bias,
        }
        for c in range(NCORES)
    ]


def kernel(x, W, b, idx, mask):
    if "nc" not in _cache:
        _cache["nc"] = _build_program()
    nc = _cache["nc"]
    in_maps = _make_in_maps({"x": x, "W": W, "b": b, "idx": idx, "mask": mask})
    res = run_bass_kernel_spmd(nc, in_maps, list(range(NCORES)))
    yp = np.concatenate([res.results[c]["y"] for c in range(NCORES)], axis=0)
    return _unpack_y(yp)
